# revision 4
# baseline (speedup 1.0000x reference)
"""AlphaNet-v1 Trainium2 kernel: windowed stats + global BatchNorm + tiny MLP.

Strategy (data-parallel over batch, 8 cores):
  Phase 1 (device): per-sample raw features (corr/cov/std/z/dec/mean/ret over
    3 windows of 10 days, plus window mean/max/min) -> raw[B,1024] in DRAM,
    plus per-group partial sums for the BatchNorm statistics.
  Host: combine partial sums -> per-column affine (A, C) for all 990 flat
    features (BN is a global affine; gamma=1 so it commutes with max/min),
    fold into W1' = W1*A, b1' = b1 + W1@C.
  Phase 2 (device): h = relu(W1' @ raw.T + b1'), y = u @ h + c0 via
    TensorEngine (PE transpose + matvec).
"""

import numpy as np

import concourse.bass as bass
import concourse.bacc as bacc
import concourse.mybir as mybir
from concourse.tile import TileContext
from concourse.bass_utils import run_bass_kernel_spmd

F32 = mybir.dt.float32
ALU = mybir.AluOpType
AX = mybir.AxisListType
AF = mybir.ActivationFunctionType

B = 131072
NCORES = 8
BS = B // NCORES            # 16384 samples per core
S = 8                       # samples per partition row per tile
PT = 128
TSAMP = PT * S              # 1024 samples per tile
NT = BS // TSAMP            # 16 tiles
NF, NW, ND = 11, 3, 10
XC = NF * NW * ND           # 330
RC = 1024                   # padded raw feature columns (990 used)
EPS = 1e-5

# base feature groups in F (165 features x 3 windows): feat ranges
GF = [(0, 55), (55, 110), (110, 121), (121, 132), (132, 143), (143, 154), (154, 165)]
GCNT = np.array([55, 55, 11, 11, 11, 11, 11], dtype=np.int64)
# (alpha, beta): true_feature = alpha*raw + beta
GAB = [(1.0, 0.0), (1.0 / 9.0, 0.0), (1.0 / 3.0, 0.0), (0.3, 0.0),
       (1.0 / 55.0, 0.0), (0.1, 0.0), (1.0, -1.0)]
# feature slices in F: corr, cov, std, z, dec, mean, ret
SL_CORR, SL_COV, SL_STD, SL_Z, SL_DEC, SL_MEAN, SL_RET = GF
# pair-block offsets for shifted products d=1..10 within 55-pair packing
OFFD = np.cumsum([0] + [11 - d for d in range(1, 11)]).tolist()  # OFFD[d-1] = start


def _feat_perm():
    """map my 165-feature index -> reference feature index (triu pair order)."""
    iu, ju = np.triu_indices(NF, k=1)
    ref_of_pair = {(i, j): k for k, (i, j) in enumerate(zip(iu, ju))}
    fmap = np.arange(165)
    for d in range(1, 11):
        for f in range(NF - d):
            mine = OFFD[d - 1] + f
            ref = ref_of_pair[(f, f + d)]
            fmap[mine] = ref           # corr block
            fmap[55 + mine] = 55 + ref  # cov block
    return fmap


FMAP = _feat_perm()
# my raw col -> reference flat col (990)
COLMAP = np.zeros(990, dtype=np.int64)
for _fm in range(165):
    for _w in range(3):
        COLMAP[3 * _fm + _w] = 3 * FMAP[_fm] + _w
for _si in range(3):
    for _fm in range(165):
        COLMAP[495 + 165 * _si + _fm] = 495 + 165 * _si + FMAP[_fm]


def build_phase1():
    nc = bacc.Bacc()
    x_in = nc.dram_tensor("x_in", [BS, XC], F32, kind="ExternalInput")
    wfull_in = nc.dram_tensor("wfull_in", [PT, S * XC], F32, kind="ExternalInput")
    raw_out = nc.dram_tensor("raw_out", [BS, RC], F32, kind="ExternalOutput")
    ps_out = nc.dram_tensor("ps_out", [PT, NT, 64], F32, kind="ExternalOutput")

    x_t = x_in.rearrange("(t p s) c -> t p (s c)", t=NT, p=PT, s=S)
    raw_t = raw_out.rearrange("(t p s) c -> t p (s c)", t=NT, p=PT, s=S)

    with TileContext(nc) as tc:
        with tc.tile_pool(name="cst", bufs=1) as cp, \
             tc.tile_pool(name="io", bufs=2) as iop, \
             tc.tile_pool(name="wk", bufs=2) as wp, \
             tc.tile_pool(name="big", bufs=2) as bp:
            wful = cp.tile([PT, S * XC], F32)
            nc.sync.dma_start(wful[:], wfull_in[:, :])
            SCR = cp.tile([PT, S * 55 * NW], F32)
            PSALL = cp.tile([PT, NT, 64], F32)
            Xt = []
            prev = None  # (RAW, PS, t)
            X0 = iop.tile([PT, S * XC], F32, tag="X")
            nc.sync.dma_start(X0[:], x_t[0])
            Xt.append(X0)
            for t in range(NT):
                if prev is not None:
                    pRAW, pt = prev
                    nc.sync.dma_start(raw_t[pt], pRAW[:])
                if t + 1 < NT:
                    Xn = iop.tile([PT, S * XC], F32, tag="X")
                    nc.sync.dma_start(Xn[:], x_t[t + 1])
                    Xt.append(Xn)
                X = Xt[t]
                Xv = X.rearrange("p (s f w d) -> p s f w d", s=S, f=NF, w=NW, d=ND)
                X4 = X.rearrange("p (s f wd) -> p s f wd", s=S, f=NF, wd=NW * ND)

                RAW = iop.tile([PT, S * RC], F32, tag="RAW")
                RV = RAW.rearrange("p (s c) -> p s c", s=S)
                F3 = RV[:, :, 0:495].rearrange("p s (f w) -> p s f w", w=NW)
                nc.vector.memset(RV[:, :, 990:RC], 0.0)

                mean3 = F3[:, :, SL_MEAN[0]:SL_MEAN[1], :]
                nc.vector.tensor_reduce(mean3, Xv, axis=AX.X, op=ALU.add)

                # squares -> s2
                SQ = bp.tile([PT, S * XC], F32, tag="BIG")
                nc.vector.tensor_tensor(SQ[:], X[:], X[:], ALU.mult)
                s2t = wp.tile([PT, S, NF, NW], F32, tag="s2t")
                nc.vector.tensor_reduce(
                    s2t[:], SQ.rearrange("p (s f w d) -> p s f w d",
                                         s=S, f=NF, w=NW, d=ND),
                    axis=AX.X, op=ALU.add)

                # shifted pair products d=1..10 -> SS [p,S,55,3]
                SS = wp.tile([PT, S, 55, NW], F32, tag="SS")
                for d in range(1, 11):
                    o = OFFD[d - 1]
                    PD = bp.tile([PT, S, NF - d, NW * ND], F32, tag="BIG")
                    nc.vector.tensor_tensor(PD[:], X4[:, :, 0:NF - d, :],
                                            X4[:, :, d:NF, :], ALU.mult)
                    nc.vector.tensor_reduce(
                        SS[:, :, o:o + NF - d, :],
                        PD.rearrange("p s f (w d) -> p s f w d", w=NW),
                        axis=AX.X, op=ALU.add)

                # scaled mean (0.1*mean') -> pre-scaled pair products
                MSC = wp.tile([PT, S, NF, NW], F32, tag="MSC")
                nc.vector.tensor_scalar_mul(MSC[:], mean3, 0.1)
                MM = wp.tile([PT, S, 55, NW], F32, tag="MM")
                for d in range(1, 11):
                    o = OFFD[d - 1]
                    nc.vector.tensor_tensor(MM[:, :, o:o + NF - d, :],
                                            MSC[:, :, 0:NF - d, :],
                                            F3[:, :, SL_MEAN[0] + d:SL_MEAN[1], :],
                                            ALU.mult)
                # var' = s2 - 0.1*mean'^2 ; cov' = SS - 0.1*MM
                VT = wp.tile([PT, S, NF, NW], F32, tag="VT")
                nc.vector.tensor_tensor(VT[:], MSC[:], mean3, ALU.mult)
                VARP = wp.tile([PT, S, NF, NW], F32, tag="VARP")
                nc.vector.tensor_tensor(VARP[:], s2t[:], VT[:], ALU.subtract)
                covF = F3[:, :, SL_COV[0]:SL_COV[1], :]
                nc.vector.tensor_tensor(covF, SS[:], MM[:], ALU.subtract)

                # std_raw = sqrt(var'), rstd = 1/std_raw
                stdF = F3[:, :, SL_STD[0]:SL_STD[1], :]
                nc.scalar.sqrt(stdF, VARP[:])
                RSTD = wp.tile([PT, S, NF, NW], F32, tag="RSTD")
                nc.vector.reciprocal(RSTD[:], stdF)

                # corr = cov' * rstd_i * rstd_j
                RR = wp.tile([PT, S, 55, NW], F32, tag="RR")
                for d in range(1, 11):
                    o = OFFD[d - 1]
                    nc.vector.tensor_tensor(RR[:, :, o:o + NF - d, :],
                                            RSTD[:, :, 0:NF - d, :],
                                            RSTD[:, :, d:NF, :], ALU.mult)
                nc.vector.tensor_tensor(F3[:, :, SL_CORR[0]:SL_CORR[1], :],
                                        covF, RR[:], ALU.mult)

                # z = mean' * rstd
                nc.vector.tensor_tensor(F3[:, :, SL_Z[0]:SL_Z[1], :],
                                        mean3, RSTD[:], ALU.mult)

                # ret = x9 / x0
                R0 = wp.tile([PT, S, NF, NW], F32, tag="R0")
                nc.vector.reciprocal(R0[:], Xv[:, :, :, :, 0])
                nc.vector.tensor_tensor(F3[:, :, SL_RET[0]:SL_RET[1], :],
                                        R0[:], Xv[:, :, :, :, 9], ALU.mult)

                # dec = sum(x * d)
                DW = bp.tile([PT, S * XC], F32, tag="BIG")
                nc.vector.tensor_tensor(DW[:], X[:], wful[:], ALU.mult)
                nc.vector.tensor_reduce(
                    F3[:, :, SL_DEC[0]:SL_DEC[1], :],
                    DW.rearrange("p (s f w d) -> p s f w d", s=S, f=NF, w=NW, d=ND),
                    axis=AX.X, op=ALU.add)

                # window mean(sum)/max/min over the 3 windows -> raw cols 495:990
                T01 = wp.tile([PT, S, 165], F32, tag="T01")
                for si, op in enumerate([ALU.add, ALU.max, ALU.min]):
                    nc.vector.tensor_tensor(T01[:], F3[:, :, :, 0], F3[:, :, :, 1], op)
                    nc.vector.tensor_tensor(RV[:, :, 495 + 165 * si:495 + 165 * (si + 1)],
                                            T01[:], F3[:, :, :, 2], op)

                # partial sums into persistent slot t
                PS = PSALL[:, t, :]
                for g, (a, b) in enumerate(GF):
                    scr = SCR[:, 0:S * (b - a) * NW].rearrange(
                        "p (s f w) -> p s f w", s=S, f=b - a, w=NW)
                    nc.scalar.activation(scr, F3[:, :, a:b, :],
                                         AF.Square, accum_out=PS[:, g:g + 1])
                for si in range(3):
                    base = 495 + 165 * si
                    for g, (a, b) in enumerate(GF):
                        seg = RV[:, :, base + a:base + b]
                        scr = SCR[:, 0:S * (b - a)].rearrange(
                            "p (s f) -> p s f", s=S, f=b - a)
                        nc.vector.tensor_reduce(PS[:, 7 + 7 * si + g:8 + 7 * si + g],
                                                seg, axis=AX.XY, op=ALU.add)
                        nc.scalar.activation(scr, seg, AF.Square,
                                             accum_out=PS[:, 28 + 7 * si + g:29 + 7 * si + g])
                prev = (RAW, t)
            pRAW, pt = prev
            nc.sync.dma_start(raw_t[pt], pRAW[:])
            nc.vector.memset(PSALL[:, :, 49:64], 0.0)
            nc.sync.dma_start(ps_out[:, :, :], PSALL[:])
    return nc


def build_phase2():
    nc = bacc.Bacc()
    raw_in = nc.dram_tensor("raw_in", [BS, RC], F32, kind="ExternalInput")
    w1t_in = nc.dram_tensor("w1t_in", [RC, 32], F32, kind="ExternalInput")
    b1_in = nc.dram_tensor("b1_in", [32, 1], F32, kind="ExternalInput")
    u_in = nc.dram_tensor("u_in", [32, 1], F32, kind="ExternalInput")
    c0_in = nc.dram_tensor("c0_in", [1, 1], F32, kind="ExternalInput")
    id_in = nc.dram_tensor("id_in", [PT, PT], F32, kind="ExternalInput")
    y_out = nc.dram_tensor("y_out", [1, BS], F32, kind="ExternalOutput")

    NB = BS // 512  # 32 blocks of 512 samples
    rb = raw_in.rearrange("(n t p) c -> n t p c", n=NB, t=4, p=PT)

    with TileContext(nc) as tc:
        with tc.tile_pool(name="cst", bufs=1) as cp, \
             tc.tile_pool(name="sb", bufs=3) as sp, \
             tc.tile_pool(name="ps", bufs=4, space="PSUM") as pp, \
             tc.tile_pool(name="ps2", bufs=2, space="PSUM") as pp2:
            W1S = cp.tile([PT, 8 * 32], F32)
            W1Sv = W1S.rearrange("p (c m) -> p c m", c=8)
            nc.sync.dma_start(W1Sv, w1t_in.rearrange("(c p) m -> p c m", c=8, p=PT))
            B1T = cp.tile([32, 1], F32)
            nc.sync.dma_start(B1T[:], b1_in[:, :])
            UT = cp.tile([32, 1], F32)
            nc.sync.dma_start(UT[:], u_in[:, :])
            C0T = cp.tile([1, 1], F32)
            nc.sync.dma_start(C0T[:], c0_in[:, :])
            IDT = cp.tile([PT, PT], F32)
            nc.sync.dma_start(IDT[:], id_in[:, :])

            for n in range(NB):
                HP = pp2.tile([32, 512], F32, tag="HP")
                for t in range(4):
                    Ft = sp.tile([PT, RC], F32, tag="Ft")
                    nc.gpsimd.dma_start(Ft[:], rb[n, t])
                    for c in range(8):
                        TP = pp.tile([PT, PT], F32, tag="TP")
                        nc.tensor.transpose(TP[:], Ft[:, c * PT:(c + 1) * PT], IDT[:])
                        FTc = sp.tile([PT, PT], F32, tag="FTc")
                        if (t * 8 + c) % 2 == 0:
                            nc.vector.tensor_copy(FTc[:], TP[:])
                        else:
                            nc.scalar.copy(FTc[:], TP[:])
                        nc.tensor.matmul(HP[:, t * PT:(t + 1) * PT], W1Sv[:, c, :],
                                         FTc[:], start=(c == 0), stop=(c == 7))
                HS = sp.tile([32, 512], F32, tag="HS")
                nc.scalar.activation(HS[:], HP[:], AF.Relu, bias=B1T[:, 0:1], scale=1.0)
                OP = pp2.tile([1, 512], F32, tag="OP")
                nc.tensor.matmul(OP[:], UT[:], HS[:], start=True, stop=True)
                OS = sp.tile([1, 512], F32, tag="OS")
                nc.vector.tensor_scalar(OS[:], OP[:], C0T[0:1, 0:1], None, ALU.add)
                nc.gpsimd.dma_start(y_out[0:1, n * 512:(n + 1) * 512], OS[:])
    return nc


_CACHE = {}
LAST_EXEC_NS = {}


def _run(nc, in_maps, **kw):
    import os
    tr = os.environ.get("KTRACE", "") == "1"
    if tr:
        kw.setdefault("trace", True)
    return run_bass_kernel_spmd(nc, in_maps, **kw)


def _get_kernels():
    if "p1" not in _CACHE:
        _CACHE["p1"] = build_phase1()
        _CACHE["p1"].finalize()
        _CACHE["p2"] = build_phase2()
        _CACHE["p2"].finalize()
    return _CACHE["p1"], _CACHE["p2"]


def kernel(x, gamma, beta, W1, b1, W2, b2, w_scale, b_scale):
    x = np.asarray(x, dtype=np.float32)
    W1 = np.asarray(W1, np.float32); b1 = np.asarray(b1, np.float32)
    W2 = np.asarray(W2, np.float32); b2 = np.asarray(b2, np.float32)
    gamma_f = float(np.asarray(gamma).reshape(-1)[0])
    beta_f = float(np.asarray(beta).reshape(-1)[0])
    wsc = float(np.asarray(w_scale).reshape(-1)[0])
    bsc = float(np.asarray(b_scale).reshape(-1)[0])

    nc1, nc2 = _get_kernels()
    xs = np.ascontiguousarray(x.reshape(B, XC))
    wbase = np.tile(np.arange(1, 11, dtype=np.float32), NF * NW)  # [330]
    wfull = np.tile(wbase, (PT, S))

    in1 = [{"x_in": xs[c * BS:(c + 1) * BS], "wfull_in": wfull} for c in range(NCORES)]
    r1 = _run(nc1, in1, core_ids=list(range(NCORES)))
    LAST_EXEC_NS["p1"] = r1.exec_time_ns
    raws = [r["raw_out"] for r in r1.results]
    P = np.zeros(64, np.float64)
    for r in r1.results:
        P += r["ps_out"].astype(np.float64).sum(axis=(0, 1))

    # base group BN affines
    A_base = np.zeros(7); C_base = np.zeros(7)
    for g in range(7):
        alpha, bet = GAB[g]
        N = float(B * GCNT[g] * 3)
        S1 = P[7 + g]          # sum of raw (= sum of wmean over group)
        S2 = P[g]              # sum of raw^2
        mT = (alpha * S1 + bet * N) / N
        e2 = (alpha * alpha * S2 + 2 * alpha * bet * S1 + bet * bet * N) / N
        v = e2 - mT * mT
        a = gamma_f / np.sqrt(v + EPS)
        c = beta_f - a * mT
        A_base[g] = a * alpha
        C_base[g] = a * bet + c

    # second-level BN affines (wmean/3, wmax, wmin; p1 = groups 0..5, p2 = {6})
    A2 = np.zeros((3, 7)); C2 = np.zeros((3, 7))
    for si in range(3):
        k = A_base * (1.0 / 3.0 if si == 0 else 1.0)
        off = C_base
        S1g = P[7 + 7 * si:14 + 7 * si].copy()
        S2g = P[28 + 7 * si:35 + 7 * si].copy()
        for grp_set, idxs in (("p1", range(6)), ("p2", [6])):
            Ntot = float(B * sum(GCNT[i] for i in idxs))
            m = sum(k[i] * S1g[i] + B * GCNT[i] * off[i] for i in idxs) / Ntot
            e2 = sum(k[i] ** 2 * S2g[i] + 2 * k[i] * off[i] * S1g[i]
                     + B * GCNT[i] * off[i] ** 2 for i in idxs) / Ntot
            v = e2 - m * m
            a2 = gamma_f / np.sqrt(v + EPS)
            c2 = beta_f - a2 * m
            for i in idxs:
                A2[si, i] = a2 * k[i]
                C2[si, i] = a2 * off[i] + c2

    # per-column affine over the 990 raw columns
    gof = np.concatenate([np.full(GCNT[g], g) for g in range(7)])  # [165] feat->group
    A = np.zeros(990); C = np.zeros(990)
    A[0:495] = np.repeat(A_base[gof], 3); C[0:495] = np.repeat(C_base[gof], 3)
    for si in range(3):
        A[495 + 165 * si:660 + 165 * si] = A2[si, gof]
        C[495 + 165 * si:660 + 165 * si] = C2[si, gof]

    W1e = W1[:, COLMAP]
    W1A = np.zeros((32, RC), np.float32)
    W1A[:30, :990] = W1e * A[None, :].astype(np.float32)
    b1p = np.zeros((32, 1), np.float32)
    b1p[:30, 0] = b1 + W1e @ C.astype(np.float32)
    u = np.zeros((32, 1), np.float32)
    u[:30, 0] = wsc * W2[0]
    c0 = np.float32(wsc * float(b2[0]) + bsc)

    in2 = [{"raw_in": raws[c], "w1t_in": np.ascontiguousarray(W1A.T),
            "b1_in": b1p, "u_in": u, "c0_in": np.array([[c0]], np.float32),
            "id_in": np.eye(PT, dtype=np.float32)} for c in range(NCORES)]
    r2 = _run(nc2, in2, core_ids=list(range(NCORES)))
    LAST_EXEC_NS["p2"] = r2.exec_time_ns
    y = np.concatenate([r["y_out"][0] for r in r2.results])
    return y.astype(np.float32)



# revision 11
# speedup vs baseline: 1.1692x; 1.1692x over previous
"""AlphaNet-v1 Trainium2 kernel: windowed stats + global BatchNorm + tiny MLP.

Strategy (data-parallel over batch, 8 cores):
  Phase 1 (device): per-sample raw features (corr/cov/std/z/dec/mean/ret over
    3 windows of 10 days, plus window sum/max/min) computed sample-major on
    DVE, then PE-transposed to feature-major rawT[1024, BS] in DRAM, plus
    per-column sum-of-squares (ScalarE accum on transposed tiles) and
    per-group linear sums for the BatchNorm statistics.
  Host: combine partial sums -> per-column affine (A, C) for all 990 flat
    features (BN is a global affine; gamma=1 so it commutes with max/min),
    fold into W1' = W1*A, b1' = b1 + W1@C.
  Phase 2 (device): h = relu(W1' @ rawT + b1') via chunked PE matmuls with
    features on the contraction axis (no transposes), y = u @ h + c0.
"""

import numpy as np

import concourse.bass as bass
import concourse.bacc as bacc
import concourse.mybir as mybir
from concourse.tile import TileContext
from concourse.bass_utils import run_bass_kernel_spmd

F32 = mybir.dt.float32
ALU = mybir.AluOpType
AX = mybir.AxisListType
AF = mybir.ActivationFunctionType

B = 131072
NCORES = 8
BS = B // NCORES            # 16384 samples per core
S = 8                       # sample-blocks per tile (each 128 samples)
PT = 128
TSAMP = PT * S              # 1024 samples per tile
NT = BS // TSAMP            # 16 tiles
NF, NW, ND = 11, 3, 10
XC = NF * NW * ND           # 330
RC = 1024                   # padded raw feature columns (990 used)
EPS = 1e-5

# base feature groups in F (165 features x 3 windows): feat ranges
GF = [(0, 55), (55, 110), (110, 121), (121, 132), (132, 143), (143, 154), (154, 165)]
GCNT = np.array([55, 55, 11, 11, 11, 11, 11], dtype=np.int64)
# (alpha, beta): true_feature = alpha*raw + beta
GAB = [(1.0, 0.0), (1.0 / 9.0, 0.0), (1.0 / 3.0, 0.0), (0.3, 0.0),
       (1.0 / 55.0, 0.0), (0.1, 0.0), (1.0, -1.0)]
# feature slices in F: corr, cov, std, z, dec, mean, ret
SL_CORR, SL_COV, SL_STD, SL_Z, SL_DEC, SL_MEAN, SL_RET = GF
# pair-block offsets for shifted products d=1..10 within 55-pair packing
OFFD = np.cumsum([0] + [11 - d for d in range(1, 11)]).tolist()  # OFFD[d-1] = start


def _feat_perm():
    """map my 165-feature index -> reference feature index (triu pair order)."""
    iu, ju = np.triu_indices(NF, k=1)
    ref_of_pair = {(i, j): k for k, (i, j) in enumerate(zip(iu, ju))}
    fmap = np.arange(165)
    for d in range(1, 11):
        for f in range(NF - d):
            mine = OFFD[d - 1] + f
            ref = ref_of_pair[(f, f + d)]
            fmap[mine] = ref           # corr block
            fmap[55 + mine] = 55 + ref  # cov block
    return fmap


FMAP = _feat_perm()
# my raw col -> reference flat col (990)
COLMAP = np.zeros(990, dtype=np.int64)
for _fm in range(165):
    for _w in range(3):
        COLMAP[3 * _fm + _w] = 3 * FMAP[_fm] + _w
for _si in range(3):
    for _fm in range(165):
        COLMAP[495 + 165 * _si + _fm] = 495 + 165 * _si + FMAP[_fm]

# my raw col -> BN group id for the squared-sum reduction (base groups 0..6,
# window-stat groups 7 + 7*si + g), -1 for pad
gof = np.concatenate([np.full(GCNT[g], g) for g in range(7)])  # feat->group
COLGRP = np.full(RC, -1, dtype=np.int64)
COLGRP[0:495] = np.repeat(gof, 3)
for _si in range(3):
    COLGRP[495 + 165 * _si:660 + 165 * _si] = 7 + 7 * _si + gof


def build_phase1():
    nc = bacc.Bacc()
    x_in = nc.dram_tensor("x_in", [BS, XC], F32, kind="ExternalInput")
    wfull_in = nc.dram_tensor("wfull_in", [PT, S * XC], F32, kind="ExternalInput")
    id_in = nc.dram_tensor("id_in", [PT, PT], F32, kind="ExternalInput")
    rawt_out = nc.dram_tensor("rawt_out", [RC, BS], F32, kind="ExternalOutput")
    ps_out = nc.dram_tensor("ps_out", [PT, NT, 64], F32, kind="ExternalOutput")
    psq_out = nc.dram_tensor("psq_out", [PT, NT, 8], F32, kind="ExternalOutput")

    # sample id = t*1024 + s*128 + p  (contiguous 128-sample runs per (t,s))
    x_t = x_in.rearrange("(t s p) c -> t p s c", t=NT, s=S, p=PT)
    # iteration order (cp, cc, j) matches the SBUF-side [p, cc, j] layout
    rt = rawt_out.rearrange("(cc cp) (t s j) -> t s cp cc j", cc=8, cp=PT, t=NT, s=S)

    with TileContext(nc) as tc:
        with tc.tile_pool(name="cst", bufs=1) as cp, \
             tc.tile_pool(name="io", bufs=2) as iop, \
             tc.tile_pool(name="wk", bufs=1) as wp, \
             tc.tile_pool(name="big", bufs=2) as bp, \
             tc.tile_pool(name="ft", bufs=1) as fp_, \
             tc.tile_pool(name="tp", bufs=2, space="PSUM") as pp:

            wful = cp.tile([PT, S * XC], F32)
            nc.sync.dma_start(wful[:], wfull_in[:, :])
            IDT = cp.tile([PT, PT], F32)
            nc.sync.dma_start(IDT[:], id_in[:, :])
            SCR = cp.tile([PT, S * 55 * NW], F32)
            PSALL = cp.tile([PT, NT, 64], F32)
            PSQ = cp.tile([PT, NT, 8], F32)
            Xt = []
            X0 = iop.tile([PT, S * XC], F32, tag="X")
            nc.sync.dma_start(X0.rearrange("p (s c) -> p s c", s=S), x_t[0])
            Xt.append(X0)
            for t in range(NT):
                if t + 1 < NT:
                    Xn = iop.tile([PT, S * XC], F32, tag="X")
                    nc.sync.dma_start(Xn.rearrange("p (s c) -> p s c", s=S),
                                      x_t[t + 1])
                    Xt.append(Xn)
                X = Xt[t]
                Xv = X.rearrange("p (s f w d) -> p s f w d", s=S, f=NF, w=NW, d=ND)
                X4 = X.rearrange("p (s f wd) -> p s f wd", s=S, f=NF, wd=NW * ND)

                RAW = fp_.tile([PT, S * RC], F32, tag="RAW")
                RV = RAW.rearrange("p (s c) -> p s c", s=S)
                F3 = RV[:, :, 0:495].rearrange("p s (f w) -> p s f w", w=NW)
                nc.vector.memset(RV[:, :, 990:RC], 0.0)

                mean3 = F3[:, :, SL_MEAN[0]:SL_MEAN[1], :]
                nc.vector.tensor_reduce(mean3, Xv, axis=AX.X, op=ALU.add)

                # squares -> s2
                SQ = bp.tile([PT, S * XC], F32, tag="BIG")
                nc.vector.tensor_tensor(SQ[:], X[:], X[:], ALU.mult)
                s2t = wp.tile([PT, S, NF, NW], F32, tag="s2t")
                nc.vector.tensor_reduce(
                    s2t[:], SQ.rearrange("p (s f w d) -> p s f w d",
                                         s=S, f=NF, w=NW, d=ND),
                    axis=AX.X, op=ALU.add)

                # shifted pair products d=1..10 -> SS [p,S,55,3]
                SS = wp.tile([PT, S, 55, NW], F32, tag="SS")
                for d in range(1, 11):
                    o = OFFD[d - 1]
                    PD = bp.tile([PT, S, NF - d, NW * ND], F32, tag="BIG")
                    nc.vector.tensor_tensor(PD[:], X4[:, :, 0:NF - d, :],
                                            X4[:, :, d:NF, :], ALU.mult)
                    nc.vector.tensor_reduce(
                        SS[:, :, o:o + NF - d, :],
                        PD.rearrange("p s f (w d) -> p s f w d", w=NW),
                        axis=AX.X, op=ALU.add)

                # scaled mean (0.1*mean') -> pre-scaled pair products
                MSC = wp.tile([PT, S, NF, NW], F32, tag="MSC")
                nc.vector.tensor_scalar_mul(MSC[:], mean3, 0.1)
                MM = wp.tile([PT, S, 55, NW], F32, tag="MM")
                for d in range(1, 11):
                    o = OFFD[d - 1]
                    nc.vector.tensor_tensor(MM[:, :, o:o + NF - d, :],
                                            MSC[:, :, 0:NF - d, :],
                                            F3[:, :, SL_MEAN[0] + d:SL_MEAN[1], :],
                                            ALU.mult)
                # var' = s2 - 0.1*mean'^2 ; cov' = SS - 0.1*MM
                VT = wp.tile([PT, S, NF, NW], F32, tag="VT")
                nc.vector.tensor_tensor(VT[:], MSC[:], mean3, ALU.mult)
                VARP = wp.tile([PT, S, NF, NW], F32, tag="VARP")
                nc.vector.tensor_tensor(VARP[:], s2t[:], VT[:], ALU.subtract)
                covF = F3[:, :, SL_COV[0]:SL_COV[1], :]
                nc.vector.tensor_tensor(covF, SS[:], MM[:], ALU.subtract)

                # std_raw = sqrt(var'), rstd = 1/std_raw
                stdF = F3[:, :, SL_STD[0]:SL_STD[1], :]
                nc.scalar.sqrt(stdF, VARP[:])
                RSTD = wp.tile([PT, S, NF, NW], F32, tag="RSTD")
                nc.vector.reciprocal(RSTD[:], stdF)

                # corr = cov' * rstd_i * rstd_j
                RR = wp.tile([PT, S, 55, NW], F32, tag="RR")
                for d in range(1, 11):
                    o = OFFD[d - 1]
                    nc.vector.tensor_tensor(RR[:, :, o:o + NF - d, :],
                                            RSTD[:, :, 0:NF - d, :],
                                            RSTD[:, :, d:NF, :], ALU.mult)
                nc.vector.tensor_tensor(F3[:, :, SL_CORR[0]:SL_CORR[1], :],
                                        covF, RR[:], ALU.mult)

                # z = mean' * rstd
                nc.vector.tensor_tensor(F3[:, :, SL_Z[0]:SL_Z[1], :],
                                        mean3, RSTD[:], ALU.mult)

                # ret = x9 / x0
                R0 = wp.tile([PT, S, NF, NW], F32, tag="R0")
                nc.vector.reciprocal(R0[:], Xv[:, :, :, :, 0])
                nc.vector.tensor_tensor(F3[:, :, SL_RET[0]:SL_RET[1], :],
                                        R0[:], Xv[:, :, :, :, 9], ALU.mult)

                # dec = sum(x * d)
                DW = bp.tile([PT, S * XC], F32, tag="BIG")
                nc.vector.tensor_tensor(DW[:], X[:], wful[:], ALU.mult)
                nc.vector.tensor_reduce(
                    F3[:, :, SL_DEC[0]:SL_DEC[1], :],
                    DW.rearrange("p (s f w d) -> p s f w d", s=S, f=NF, w=NW, d=ND),
                    axis=AX.X, op=ALU.add)

                # window sum/max/min over the 3 windows -> raw cols 495:990
                T01 = wp.tile([PT, S, 165], F32, tag="T01")
                for si, op in enumerate([ALU.add, ALU.max, ALU.min]):
                    nc.vector.tensor_tensor(T01[:], F3[:, :, :, 0], F3[:, :, :, 1], op)
                    nc.vector.tensor_tensor(RV[:, :, 495 + 165 * si:495 + 165 * (si + 1)],
                                            T01[:], F3[:, :, :, 2], op)

                # linear partial sums per group (DVE, per-partition partials)
                PS = PSALL[:, t, :]
                for si in range(3):
                    base = 495 + 165 * si
                    for g, (a, b) in enumerate(GF):
                        seg = RV[:, :, base + a:base + b]
                        nc.vector.tensor_reduce(PS[:, 7 + 7 * si + g:8 + 7 * si + g],
                                                seg, axis=AX.XY, op=ALU.add)

                # transpose RAW -> feature-major, evac, per-column sq-sums, DMA
                FT = fp_.tile([PT, S, 8, PT], F32, tag="FT")
                for s in range(S):
                    TP = pp.tile([PT, 8 * PT], F32, tag="TP")
                    for cc in range(8):
                        nc.tensor.transpose(TP[:, cc * PT:(cc + 1) * PT],
                                            RV[:, s, cc * PT:(cc + 1) * PT], IDT[:])
                    nc.scalar.copy(FT[:, s, :, :].rearrange("p a b -> p (a b)"), TP[:])
                    nc.sync.dma_start(rt[t, s], FT[:, s, :, :])
                for cc in range(8):
                    scr = SCR[:, 0:S * PT].rearrange("p (s j) -> p s j", s=S)
                    nc.scalar.activation(scr, FT[:, :, cc, :], AF.Square,
                                         accum_out=PSQ[:, t, cc:cc + 1])
            nc.vector.memset(PSALL[:, :, 0:7], 0.0)
            nc.vector.memset(PSALL[:, :, 28:64], 0.0)
            nc.sync.dma_start(ps_out[:, :, :], PSALL[:])
            nc.sync.dma_start(psq_out[:, :, :], PSQ[:])
    return nc


def build_phase2():
    nc = bacc.Bacc()
    rawt_in = nc.dram_tensor("rawt_in", [RC, BS], F32, kind="ExternalInput")
    w1t_in = nc.dram_tensor("w1t_in", [RC, 32], F32, kind="ExternalInput")
    b1_in = nc.dram_tensor("b1_in", [32, 1], F32, kind="ExternalInput")
    u_in = nc.dram_tensor("u_in", [32, 1], F32, kind="ExternalInput")
    c0_in = nc.dram_tensor("c0_in", [1, 1], F32, kind="ExternalInput")
    y_out = nc.dram_tensor("y_out", [1, BS], F32, kind="ExternalOutput")

    NB = BS // 512  # 32 blocks of 512 samples
    rtb = rawt_in.rearrange("(cc cp) (n j) -> n cp cc j", cc=8, cp=PT, n=NB)

    with TileContext(nc) as tc:
        with tc.tile_pool(name="cst", bufs=1) as cp, \
             tc.tile_pool(name="sb", bufs=3) as sp, \
             tc.tile_pool(name="ps", bufs=4, space="PSUM") as pp:
            W1S = cp.tile([PT, 8 * 32], F32)
            W1Sv = W1S.rearrange("p (c m) -> p c m", c=8)
            nc.sync.dma_start(W1Sv, w1t_in.rearrange("(c p) m -> p c m", c=8, p=PT))
            B1T = cp.tile([32, 1], F32)
            nc.sync.dma_start(B1T[:], b1_in[:, :])
            UT = cp.tile([32, 1], F32)
            nc.sync.dma_start(UT[:], u_in[:, :])
            C0T = cp.tile([1, 1], F32)
            nc.sync.dma_start(C0T[:], c0_in[:, :])

            for n in range(NB):
                RT = sp.tile([PT, 8, 512], F32, tag="RT")
                nc.gpsimd.dma_start(RT[:], rtb[n])
                HP = pp.tile([32, 512], F32, tag="HP")
                for cc in range(8):
                    nc.tensor.matmul(HP[:], W1Sv[:, cc, :], RT[:, cc, :],
                                     start=(cc == 0), stop=(cc == 7))
                HS = sp.tile([32, 512], F32, tag="HS")
                nc.scalar.activation(HS[:], HP[:], AF.Relu, bias=B1T[:, 0:1], scale=1.0)
                OP = pp.tile([1, 512], F32, tag="OP")
                nc.tensor.matmul(OP[:], UT[:], HS[:], start=True, stop=True)
                OS = sp.tile([1, 512], F32, tag="OS")
                nc.vector.tensor_scalar(OS[:], OP[:], C0T[0:1, 0:1], None, ALU.add)
                nc.gpsimd.dma_start(y_out[0:1, n * 512:(n + 1) * 512], OS[:])
    return nc


_CACHE = {}
LAST_EXEC_NS = {}


def _run(nc, in_maps, **kw):
    import os
    tr = os.environ.get("KTRACE", "") == "1"
    if tr:
        kw.setdefault("trace", True)
    return run_bass_kernel_spmd(nc, in_maps, **kw)


def _get_kernels():
    if "p1" not in _CACHE:
        _CACHE["p1"] = build_phase1()
        _CACHE["p1"].finalize()
        _CACHE["p2"] = build_phase2()
        _CACHE["p2"].finalize()
    return _CACHE["p1"], _CACHE["p2"]


def kernel(x, gamma, beta, W1, b1, W2, b2, w_scale, b_scale):
    x = np.asarray(x, dtype=np.float32)
    W1 = np.asarray(W1, np.float32); b1 = np.asarray(b1, np.float32)
    W2 = np.asarray(W2, np.float32); b2 = np.asarray(b2, np.float32)
    gamma_f = float(np.asarray(gamma).reshape(-1)[0])
    beta_f = float(np.asarray(beta).reshape(-1)[0])
    wsc = float(np.asarray(w_scale).reshape(-1)[0])
    bsc = float(np.asarray(b_scale).reshape(-1)[0])

    nc1, nc2 = _get_kernels()
    xs = np.ascontiguousarray(x.reshape(B, XC))
    wbase = np.tile(np.arange(1, 11, dtype=np.float32), NF * NW)  # [330]
    wfull = np.tile(wbase, (PT, S))
    ident = np.eye(PT, dtype=np.float32)

    in1 = [{"x_in": xs[c * BS:(c + 1) * BS], "wfull_in": wfull, "id_in": ident}
           for c in range(NCORES)]
    r1 = _run(nc1, in1, core_ids=list(range(NCORES)))
    LAST_EXEC_NS["p1"] = r1.exec_time_ns
    rawts = [r["rawt_out"] for r in r1.results]
    P = np.zeros(64, np.float64)
    for r in r1.results:
        P += r["ps_out"].astype(np.float64).sum(axis=(0, 1))
    # per-column squared sums -> group squared sums
    csq = np.zeros(RC, np.float64)
    for r in r1.results:
        # psq_out[p, t, cc] = sum over (s,j) of rawT[cc*128+p, ...]^2
        psq = r["psq_out"].astype(np.float64).sum(axis=1)  # [128, 8]
        csq += psq.T.reshape(-1)                           # col = cc*128+p
    for g in range(28):
        P[g if g < 7 else 21 + g] = csq[COLGRP == g].sum()

    # base group BN affines
    A_base = np.zeros(7); C_base = np.zeros(7)
    for g in range(7):
        alpha, bet = GAB[g]
        N = float(B * GCNT[g] * 3)
        S1 = P[7 + g]          # sum of raw (= sum of wsum over group)
        S2 = P[g]              # sum of raw^2
        mT = (alpha * S1 + bet * N) / N
        e2 = (alpha * alpha * S2 + 2 * alpha * bet * S1 + bet * bet * N) / N
        v = e2 - mT * mT
        a = gamma_f / np.sqrt(v + EPS)
        c = beta_f - a * mT
        A_base[g] = a * alpha
        C_base[g] = a * bet + c

    # second-level BN affines (wsum/3, wmax, wmin; p1 = groups 0..5, p2 = {6})
    A2 = np.zeros((3, 7)); C2 = np.zeros((3, 7))
    for si in range(3):
        k = A_base * (1.0 / 3.0 if si == 0 else 1.0)
        off = C_base
        S1g = P[7 + 7 * si:14 + 7 * si].copy()
        S2g = P[28 + 7 * si:35 + 7 * si].copy()
        for grp_set, idxs in (("p1", range(6)), ("p2", [6])):
            Ntot = float(B * sum(GCNT[i] for i in idxs))
            m = sum(k[i] * S1g[i] + B * GCNT[i] * off[i] for i in idxs) / Ntot
            e2 = sum(k[i] ** 2 * S2g[i] + 2 * k[i] * off[i] * S1g[i]
                     + B * GCNT[i] * off[i] ** 2 for i in idxs) / Ntot
            v = e2 - m * m
            a2 = gamma_f / np.sqrt(v + EPS)
            c2 = beta_f - a2 * m
            for i in idxs:
                A2[si, i] = a2 * k[i]
                C2[si, i] = a2 * off[i] + c2

    # per-column affine over the 990 raw columns
    A = np.zeros(990); C = np.zeros(990)
    A[0:495] = np.repeat(A_base[gof], 3); C[0:495] = np.repeat(C_base[gof], 3)
    for si in range(3):
        A[495 + 165 * si:660 + 165 * si] = A2[si, gof]
        C[495 + 165 * si:660 + 165 * si] = C2[si, gof]

    W1e = W1[:, COLMAP]
    W1A = np.zeros((32, RC), np.float32)
    W1A[:30, :990] = W1e * A[None, :].astype(np.float32)
    b1p = np.zeros((32, 1), np.float32)
    b1p[:30, 0] = b1 + W1e @ C.astype(np.float32)
    u = np.zeros((32, 1), np.float32)
    u[:30, 0] = wsc * W2[0]
    c0 = np.float32(wsc * float(b2[0]) + bsc)

    in2 = [{"rawt_in": rawts[c], "w1t_in": np.ascontiguousarray(W1A.T),
            "b1_in": b1p, "u_in": u, "c0_in": np.array([[c0]], np.float32)}
           for c in range(NCORES)]
    r2 = _run(nc2, in2, core_ids=list(range(NCORES)))
    LAST_EXEC_NS["p2"] = r2.exec_time_ns
    # sample id within core = t*1024 + s*128 + p == linear index (identity)
    y = np.concatenate([r["y_out"][0] for r in r2.results])
    return y.astype(np.float32)


# revision 26
# speedup vs baseline: 1.1989x; 1.0254x over previous
"""AlphaNet-v1 Trainium2 kernel: windowed stats + global BatchNorm + tiny MLP.

Strategy (data-parallel over batch, 8 cores):
  Phase 1 (device): per-sample raw features (corr/cov/std/z/dec/mean/ret over
    3 windows of 10 days, plus window sum/max/min) computed sample-major on
    DVE, then PE-transposed to feature-major rawT[1024, BS] in DRAM, plus
    per-column sum-of-squares (ScalarE accum on transposed tiles) and
    per-group linear sums for the BatchNorm statistics.
  Host: combine partial sums -> per-column affine (A, C) for all 990 flat
    features (BN is a global affine; gamma=1 so it commutes with max/min),
    fold into W1' = W1*A, b1' = b1 + W1@C.
  Phase 2 (device): h = relu(W1' @ rawT + b1') via chunked PE matmuls with
    features on the contraction axis (no transposes), y = u @ h + c0.
"""

import numpy as np

import concourse.bass as bass
import concourse.bacc as bacc
import concourse.mybir as mybir
from concourse.tile import TileContext
from concourse.bass_utils import run_bass_kernel_spmd
from concourse.dve_spec import Spec, Src0, Src1, C0, AluOp
from concourse.dve_spec import scan as dve_scan, lower as dve_lower
from concourse.dve_uop import DveOpSpec
from concourse import dve_ops as _dvo


def _get_prodsum():
    """Custom DVE op: out[k] = s0 + sum_{j<=k} in0[j]*in1[j] (chained prefix
    of products). Window sums of products fall out as strided differences."""
    for o in _dvo.OPS:
        if o.name == "PRODSUM_AN":
            return o
    spec = Spec(
        body=dve_scan(AluOp.ADD, Src0 * Src1, init=C0),
        reference=lambda in0, in1, s0, s1, imm2: (
            np.cumsum((in0 * in1).astype(np.float64), axis=-1)
            + np.asarray(s0, np.float64).reshape(-1, 1)
        ).astype(np.float32),
    )
    shas = {v: DveOpSpec(name="PRODSUM_AN", uops=dve_lower(spec, ver=v),
                         rd1_en=True).sha(v) for v in ("v3", "v4")}
    op = _dvo.DveOp("PRODSUM_AN", spec, subdim=False, uops_sha=shas)
    _dvo.OPS.append(op)
    _dvo.CUSTOM_DVE_SPECS[op.name] = op.spec
    _dvo._SUB_OPCODE_FOR_NAME[op.name] = (
        _dvo._CUSTOM_DVE_ROW_BASE + len(_dvo.OPS) - 1)
    return op


PRODSUM = _get_prodsum()

F32 = mybir.dt.float32
ALU = mybir.AluOpType
AX = mybir.AxisListType
AF = mybir.ActivationFunctionType

B = 131072
NCORES = 8
BS = B // NCORES            # 16384 samples per core
S = 8                       # sample-blocks per tile (each 128 samples)
PT = 128
TSAMP = PT * S              # 1024 samples per tile
NT = BS // TSAMP            # 16 tiles
NF, NW, ND = 11, 3, 10
XC = NF * NW * ND           # 330
RC = 1024                   # padded raw feature columns (990 used)
EPS = 1e-5

# base feature groups in F (165 features x 3 windows): feat ranges
GF = [(0, 55), (55, 110), (110, 121), (121, 132), (132, 143), (143, 154), (154, 165)]
GCNT = np.array([55, 55, 11, 11, 11, 11, 11], dtype=np.int64)
# (alpha, beta): true_feature = alpha*raw + beta
GAB = [(1.0, 0.0), (1.0 / 9.0, 0.0), (1.0 / 3.0, 0.0), (0.3, 0.0),
       (1.0 / 55.0, 0.0), (0.1, 0.0), (1.0, -1.0)]
# feature slices in F: corr, cov, std, z, dec, mean, ret
SL_CORR, SL_COV, SL_STD, SL_Z, SL_DEC, SL_MEAN, SL_RET = GF
# pair-block offsets for shifted products d=1..10 within 55-pair packing
OFFD = np.cumsum([0] + [11 - d for d in range(1, 11)]).tolist()  # OFFD[d-1] = start


def _feat_perm():
    """map my 165-feature index -> reference feature index (triu pair order)."""
    iu, ju = np.triu_indices(NF, k=1)
    ref_of_pair = {(i, j): k for k, (i, j) in enumerate(zip(iu, ju))}
    fmap = np.arange(165)
    for d in range(1, 11):
        for f in range(NF - d):
            mine = OFFD[d - 1] + f
            ref = ref_of_pair[(f, f + d)]
            fmap[mine] = ref           # corr block
            fmap[55 + mine] = 55 + ref  # cov block
    return fmap


FMAP = _feat_perm()
# my raw col -> reference flat col (990)
COLMAP = np.zeros(990, dtype=np.int64)
for _fm in range(165):
    for _w in range(3):
        COLMAP[3 * _fm + _w] = 3 * FMAP[_fm] + _w
for _si in range(3):
    for _fm in range(165):
        COLMAP[495 + 165 * _si + _fm] = 495 + 165 * _si + FMAP[_fm]

# my raw col -> BN group id for the squared-sum reduction (base groups 0..6,
# window-stat groups 7 + 7*si + g), -1 for pad
gof = np.concatenate([np.full(GCNT[g], g) for g in range(7)])  # feat->group
COLGRP = np.full(RC, -1, dtype=np.int64)
COLGRP[0:495] = np.repeat(gof, 3)
for _si in range(3):
    COLGRP[495 + 165 * _si:660 + 165 * _si] = 7 + 7 * _si + gof


def build_phase1():
    nc = bacc.Bacc()
    x_in = nc.dram_tensor("x_in", [BS, XC], F32, kind="ExternalInput")
    wfull_in = nc.dram_tensor("wfull_in", [PT, S * XC], F32, kind="ExternalInput")
    id_in = nc.dram_tensor("id_in", [PT, PT], F32, kind="ExternalInput")
    rawt_out = nc.dram_tensor("rawt_out", [RC, BS], F32, kind="ExternalOutput")
    psq_out = nc.dram_tensor("psq_out", [PT, NT, 8], F32, kind="ExternalOutput")
    ps_out = nc.dram_tensor("ps_out", [PT, NT, 32], F32, kind="ExternalOutput")

    # sample id = t*1024 + s*128 + p  (contiguous 128-sample runs per (t,s))
    x_t = x_in.rearrange("(t s p) c -> t p s c", t=NT, s=S, p=PT)
    # iteration order (cp, cc, j) matches the SBUF-side [p, cc, j] layout
    rt = rawt_out.rearrange("(cc cp) (t s j) -> t s cp cc j", cc=8, cp=PT, t=NT, s=S)

    with TileContext(nc) as tc:
        with tc.tile_pool(name="cst", bufs=1) as cp, \
             tc.tile_pool(name="io", bufs=2) as iop, \
             tc.tile_pool(name="wk", bufs=1) as wp, \
             tc.tile_pool(name="big", bufs=2) as bp, \
             tc.tile_pool(name="ft", bufs=1) as fp_, \
             tc.tile_pool(name="tp", bufs=2, space="PSUM") as pp:

            wful = cp.tile([PT, S * XC], F32)
            nc.sync.dma_start(wful[:], wfull_in[:, :])
            IDT = cp.tile([PT, PT], F32)
            nc.sync.dma_start(IDT[:], id_in[:, :])
            SCR = cp.tile([PT, S * 55 * NW], F32)
            PSQ = cp.tile([PT, NT, 8], F32)
            PSALL = cp.tile([PT, NT, 32], F32)
            Xt = []
            X0 = iop.tile([PT, S * XC], F32, tag="X")
            nc.sync.dma_start(X0.rearrange("p (s c) -> p s c", s=S), x_t[0])
            Xt.append(X0)
            for t in range(NT):
                if t + 1 < NT:
                    Xn = iop.tile([PT, S * XC], F32, tag="X")
                    nc.sync.dma_start(Xn.rearrange("p (s c) -> p s c", s=S),
                                      x_t[t + 1])
                    Xt.append(Xn)
                X = Xt[t]
                Xv = X.rearrange("p (s f w d) -> p s f w d", s=S, f=NF, w=NW, d=ND)
                X4 = X.rearrange("p (s f wd) -> p s f wd", s=S, f=NF, wd=NW * ND)

                RAW = fp_.tile([PT, S * RC], F32, tag="RAW")
                RV = RAW.rearrange("p (s c) -> p s c", s=S)
                F3 = RV[:, :, 0:495].rearrange("p s (f w) -> p s f w", w=NW)
                nc.vector.memset(RV[:, :, 990:RC], 0.0)

                mean3 = F3[:, :, SL_MEAN[0]:SL_MEAN[1], :]
                nc.vector.tensor_reduce(mean3, Xv, axis=AX.X, op=ALU.add)

                # squares -> s2 (squares on ScalarE, reduce on DVE)
                SQ = bp.tile([PT, S * XC], F32, tag="BIG")
                nc.scalar.activation(SQ[:], X[:], AF.Square)
                s2t = wp.tile([PT, S, NF, NW], F32, tag="s2t")
                nc.vector.tensor_reduce(
                    s2t[:], SQ.rearrange("p (s f w d) -> p s f w d",
                                         s=S, f=NF, w=NW, d=ND),
                    axis=AX.X, op=ALU.add)

                # shifted pair products d=1..10 -> SS [p,S,55,3]
                SS = wp.tile([PT, S, 55, NW], F32, tag="SS")
                for d in range(1, 11):
                    o = OFFD[d - 1]
                    PD = bp.tile([PT, S, NF - d, NW * ND], F32, tag="BIG")
                    nc.vector.tensor_tensor(PD[:], X4[:, :, 0:NF - d, :],
                                            X4[:, :, d:NF, :], ALU.mult)
                    nc.vector.tensor_reduce(
                        SS[:, :, o:o + NF - d, :],
                        PD.rearrange("p s f (w d) -> p s f w d", w=NW),
                        axis=AX.X, op=ALU.add)

                # scaled mean (0.1*mean') -> pre-scaled pair products
                MSC = wp.tile([PT, S, NF, NW], F32, tag="MSC")
                nc.vector.tensor_scalar_mul(MSC[:], mean3, 0.1)
                MM = wp.tile([PT, S, 55, NW], F32, tag="MM")
                for d in range(1, 11):
                    o = OFFD[d - 1]
                    nc.vector.tensor_tensor(MM[:, :, o:o + NF - d, :],
                                            MSC[:, :, 0:NF - d, :],
                                            F3[:, :, SL_MEAN[0] + d:SL_MEAN[1], :],
                                            ALU.mult)
                # var' = s2 - 0.1*mean'^2 ; cov' = SS - 0.1*MM
                VT = wp.tile([PT, S, NF, NW], F32, tag="VT")
                nc.vector.tensor_tensor(VT[:], MSC[:], mean3, ALU.mult)
                VARP = wp.tile([PT, S, NF, NW], F32, tag="VARP")
                nc.vector.tensor_tensor(VARP[:], s2t[:], VT[:], ALU.subtract)
                covF = F3[:, :, SL_COV[0]:SL_COV[1], :]
                nc.vector.tensor_tensor(covF, SS[:], MM[:], ALU.subtract)

                # dec = sum(x * d)
                DW = bp.tile([PT, S * XC], F32, tag="BIG")
                nc.vector.tensor_tensor(DW[:], X[:], wful[:], ALU.mult)
                nc.vector.tensor_reduce(
                    F3[:, :, SL_DEC[0]:SL_DEC[1], :],
                    DW.rearrange("p (s f w d) -> p s f w d", s=S, f=NF, w=NW, d=ND),
                    axis=AX.X, op=ALU.add)

                # std_raw = sqrt(var'), rstd = 1/std_raw
                stdF = F3[:, :, SL_STD[0]:SL_STD[1], :]
                nc.scalar.sqrt(stdF, VARP[:])
                RSTD = wp.tile([PT, S, NF, NW], F32, tag="RSTD")
                nc.vector.reciprocal(RSTD[:], stdF)

                # corr = cov' * rstd_i * rstd_j
                RR = wp.tile([PT, S, 55, NW], F32, tag="RR")
                for d in range(1, 11):
                    o = OFFD[d - 1]
                    nc.vector.tensor_tensor(RR[:, :, o:o + NF - d, :],
                                            RSTD[:, :, 0:NF - d, :],
                                            RSTD[:, :, d:NF, :], ALU.mult)
                nc.vector.tensor_tensor(F3[:, :, SL_CORR[0]:SL_CORR[1], :],
                                        covF, RR[:], ALU.mult)

                # z = mean' * rstd
                nc.vector.tensor_tensor(F3[:, :, SL_Z[0]:SL_Z[1], :],
                                        mean3, RSTD[:], ALU.mult)

                # ret = x9 / x0
                R0 = wp.tile([PT, S, NF, NW], F32, tag="R0")
                nc.vector.reciprocal(R0[:], Xv[:, :, :, :, 0])
                nc.vector.tensor_tensor(F3[:, :, SL_RET[0]:SL_RET[1], :],
                                        R0[:], Xv[:, :, :, :, 9], ALU.mult)

                # window sum/max/min over the 3 windows -> raw cols 495:990
                T01 = wp.tile([PT, S, 165], F32, tag="T01")
                for si, op in enumerate([ALU.add, ALU.max, ALU.min]):
                    nc.vector.tensor_tensor(T01[:], F3[:, :, :, 0], F3[:, :, :, 1], op)
                    nc.vector.tensor_tensor(RV[:, :, 495 + 165 * si:495 + 165 * (si + 1)],
                                            T01[:], F3[:, :, :, 2], op)

                # linear partial sums per group (DVE, per-partition partials)
                PS = PSALL[:, t, :]
                for si in range(3):
                    base = 495 + 165 * si
                    for g, (a, b) in enumerate(GF):
                        seg = RV[:, :, base + a:base + b]
                        nc.vector.tensor_reduce(PS[:, 7 * si + g:7 * si + g + 1],
                                                seg, axis=AX.XY, op=ALU.add)

                # transpose RAW -> feature-major, evac, per-column sums, DMA
                FT = fp_.tile([PT, S, 8, PT], F32, tag="FT")
                for s in range(S):
                    TP = pp.tile([PT, 8 * PT], F32, tag="TP")
                    for cc in range(8):
                        nc.tensor.transpose(TP[:, cc * PT:(cc + 1) * PT],
                                            RV[:, s, cc * PT:(cc + 1) * PT], IDT[:])
                    nc.scalar.copy(FT[:, s, :, :].rearrange("p a b -> p (a b)"), TP[:])
                    nc.sync.dma_start(rt[t, s], FT[:, s, :, :])
                for cc in range(8):
                    scr = SCR[:, 0:S * PT].rearrange("p (s j) -> p s j", s=S)
                    nc.scalar.activation(scr, FT[:, :, cc, :], AF.Square,
                                         accum_out=PSQ[:, t, cc:cc + 1])
            nc.vector.memset(PSALL[:, :, 21:32], 0.0)
            nc.sync.dma_start(psq_out[:, :, :], PSQ[:])
            nc.sync.dma_start(ps_out[:, :, :], PSALL[:])
    return nc


def build_phase2():
    nc = bacc.Bacc()
    rawt_in = nc.dram_tensor("rawt_in", [RC, BS], F32, kind="ExternalInput")
    w1t_in = nc.dram_tensor("w1t_in", [RC, 32], F32, kind="ExternalInput")
    b1_in = nc.dram_tensor("b1_in", [32, 1], F32, kind="ExternalInput")
    u_in = nc.dram_tensor("u_in", [32, 1], F32, kind="ExternalInput")
    c0_in = nc.dram_tensor("c0_in", [1, 1], F32, kind="ExternalInput")
    y_out = nc.dram_tensor("y_out", [1, BS], F32, kind="ExternalOutput")

    NB = BS // 512  # 32 blocks of 512 samples
    rtb = rawt_in.rearrange("(cc cp) (n j) -> n cp cc j", cc=8, cp=PT, n=NB)

    with TileContext(nc) as tc:
        with tc.tile_pool(name="cst", bufs=1) as cp, \
             tc.tile_pool(name="sb", bufs=3) as sp, \
             tc.tile_pool(name="ps", bufs=4, space="PSUM") as pp:
            W1S = cp.tile([PT, 8 * 32], F32)
            W1Sv = W1S.rearrange("p (c m) -> p c m", c=8)
            nc.sync.dma_start(W1Sv, w1t_in.rearrange("(c p) m -> p c m", c=8, p=PT))
            B1T = cp.tile([32, 1], F32)
            nc.sync.dma_start(B1T[:], b1_in[:, :])
            UT = cp.tile([32, 1], F32)
            nc.sync.dma_start(UT[:], u_in[:, :])
            C0T = cp.tile([1, 1], F32)
            nc.sync.dma_start(C0T[:], c0_in[:, :])

            for n in range(NB):
                RT = sp.tile([PT, 8, 512], F32, tag="RT")
                nc.gpsimd.dma_start(RT[:], rtb[n])
                HP = pp.tile([32, 512], F32, tag="HP")
                for cc in range(8):
                    nc.tensor.matmul(HP[:], W1Sv[:, cc, :], RT[:, cc, :],
                                     start=(cc == 0), stop=(cc == 7))
                HS = sp.tile([32, 512], F32, tag="HS")
                nc.scalar.activation(HS[:], HP[:], AF.Relu, bias=B1T[:, 0:1], scale=1.0)
                OP = pp.tile([1, 512], F32, tag="OP")
                nc.tensor.matmul(OP[:], UT[:], HS[:], start=True, stop=True)
                OS = sp.tile([1, 512], F32, tag="OS")
                nc.vector.tensor_scalar(OS[:], OP[:], C0T[0:1, 0:1], None, ALU.add)
                nc.gpsimd.dma_start(y_out[0:1, n * 512:(n + 1) * 512], OS[:])
    return nc


_CACHE = {}
LAST_EXEC_NS = {}


def _run(nc, in_maps, **kw):
    import os
    tr = os.environ.get("KTRACE", "") == "1"
    if tr:
        kw.setdefault("trace", True)
    return run_bass_kernel_spmd(nc, in_maps, **kw)


def _get_kernels():
    if "p1" not in _CACHE:
        _CACHE["p1"] = build_phase1()
        _CACHE["p1"].finalize()
        _CACHE["p2"] = build_phase2()
        _CACHE["p2"].finalize()
    return _CACHE["p1"], _CACHE["p2"]


def kernel(x, gamma, beta, W1, b1, W2, b2, w_scale, b_scale):
    x = np.asarray(x, dtype=np.float32)
    W1 = np.asarray(W1, np.float32); b1 = np.asarray(b1, np.float32)
    W2 = np.asarray(W2, np.float32); b2 = np.asarray(b2, np.float32)
    gamma_f = float(np.asarray(gamma).reshape(-1)[0])
    beta_f = float(np.asarray(beta).reshape(-1)[0])
    wsc = float(np.asarray(w_scale).reshape(-1)[0])
    bsc = float(np.asarray(b_scale).reshape(-1)[0])

    nc1, nc2 = _get_kernels()
    xs = np.ascontiguousarray(x.reshape(B, XC))
    wbase = np.tile(np.arange(1, 11, dtype=np.float32), NF * NW)  # [330]
    wfull = np.tile(wbase, (PT, S))
    ident = np.eye(PT, dtype=np.float32)

    in1 = [{"x_in": xs[c * BS:(c + 1) * BS], "wfull_in": wfull, "id_in": ident}
           for c in range(NCORES)]
    r1 = _run(nc1, in1, core_ids=list(range(NCORES)))
    LAST_EXEC_NS["p1"] = r1.exec_time_ns
    rawts = [r["rawt_out"] for r in r1.results]
    # per-column squared sums (col id = cc*128 + p) + per-group linear sums
    csq = np.zeros(RC, np.float64)
    PL = np.zeros(32, np.float64)
    for r in r1.results:
        csq += r["psq_out"].astype(np.float64).sum(axis=1).T.reshape(-1)
        PL += r["ps_out"].astype(np.float64).sum(axis=(0, 1))
    P = np.zeros(64, np.float64)
    P[7:28] = PL[0:21]
    for g in range(28):
        P[g if g < 7 else 21 + g] = csq[COLGRP == g].sum()

    # base group BN affines
    A_base = np.zeros(7); C_base = np.zeros(7)
    for g in range(7):
        alpha, bet = GAB[g]
        N = float(B * GCNT[g] * 3)
        S1 = P[7 + g]          # sum of raw (= sum of wsum over group)
        S2 = P[g]              # sum of raw^2
        mT = (alpha * S1 + bet * N) / N
        e2 = (alpha * alpha * S2 + 2 * alpha * bet * S1 + bet * bet * N) / N
        v = e2 - mT * mT
        a = gamma_f / np.sqrt(v + EPS)
        c = beta_f - a * mT
        A_base[g] = a * alpha
        C_base[g] = a * bet + c

    # second-level BN affines (wsum/3, wmax, wmin; p1 = groups 0..5, p2 = {6})
    A2 = np.zeros((3, 7)); C2 = np.zeros((3, 7))
    for si in range(3):
        k = A_base * (1.0 / 3.0 if si == 0 else 1.0)
        off = C_base
        S1g = P[7 + 7 * si:14 + 7 * si].copy()
        S2g = P[28 + 7 * si:35 + 7 * si].copy()
        for grp_set, idxs in (("p1", range(6)), ("p2", [6])):
            Ntot = float(B * sum(GCNT[i] for i in idxs))
            m = sum(k[i] * S1g[i] + B * GCNT[i] * off[i] for i in idxs) / Ntot
            e2 = sum(k[i] ** 2 * S2g[i] + 2 * k[i] * off[i] * S1g[i]
                     + B * GCNT[i] * off[i] ** 2 for i in idxs) / Ntot
            v = e2 - m * m
            a2 = gamma_f / np.sqrt(v + EPS)
            c2 = beta_f - a2 * m
            for i in idxs:
                A2[si, i] = a2 * k[i]
                C2[si, i] = a2 * off[i] + c2

    # per-column affine over the 990 raw columns
    A = np.zeros(990); C = np.zeros(990)
    A[0:495] = np.repeat(A_base[gof], 3); C[0:495] = np.repeat(C_base[gof], 3)
    for si in range(3):
        A[495 + 165 * si:660 + 165 * si] = A2[si, gof]
        C[495 + 165 * si:660 + 165 * si] = C2[si, gof]

    W1e = W1[:, COLMAP]
    W1A = np.zeros((32, RC), np.float32)
    W1A[:30, :990] = W1e * A[None, :].astype(np.float32)
    b1p = np.zeros((32, 1), np.float32)
    b1p[:30, 0] = b1 + W1e @ C.astype(np.float32)
    u = np.zeros((32, 1), np.float32)
    u[:30, 0] = wsc * W2[0]
    c0 = np.float32(wsc * float(b2[0]) + bsc)

    in2 = [{"rawt_in": rawts[c], "w1t_in": np.ascontiguousarray(W1A.T),
            "b1_in": b1p, "u_in": u, "c0_in": np.array([[c0]], np.float32)}
           for c in range(NCORES)]
    r2 = _run(nc2, in2, core_ids=list(range(NCORES)))
    LAST_EXEC_NS["p2"] = r2.exec_time_ns
    # sample id within core = t*1024 + s*128 + p == linear index (identity)
    y = np.concatenate([r["y_out"][0] for r in r2.results])
    return y.astype(np.float32)
